# revision 56
# baseline (speedup 1.0000x reference)
"""CARAFE-downsample (K=5, stride=2) Trainium2 kernel, 8-core SPMD.

Key ideas:
- Host de-interleaves x into 4 parity subgrids (fp16) so every
  stride-2 access (conv3x3 taps + all 25 reassembly taps) becomes a
  contiguous slice.
- Both 128-channel chips live in ONE SBUF tile (chip dim inside the
  free dims), so each reassembly mult/add is a single FD=2048 DVE op
  instead of two FD=1024 ops: 49 ops/chunk instead of 98, amortizing
  the ~160ns/op DVE overhead.
- conv+softmax mask production is split into 4 row-slabs (8 output
  rows each) and issued ahead of reassembly (A0 A1 R0 A2 A3 R1), so
  the serial head is only two slabs deep and chunk-1 masks are
  produced while chunk-0 reassembly runs.
- Masks bounce through DRAM per half-chunk and are broadcast to 128
  partitions by replicating DMAs; mb pool slots auto-throttle the
  prefetch of chunk-1 masks during chunk-0 reassembly.
- Reassembly adds use two independent accumulator chains to avoid
  back-to-back RAW stalls on the DVE.
- Output stays fp16 and is DMA'd straight from the accumulator
  (host upcasts); mask-softmax path stays fp32 (the 16-way channel
  product amplifies logit errors).
Sharding: core = batch*2 + H-half; zero-padded 2-row/2-col halo.
"""

import numpy as np

import concourse.bacc as bacc
import concourse.mybir as mybir
import concourse.tile as tile

F32 = mybir.dt.float32
F16 = mybir.dt.float16
AX = mybir.AxisListType
OP = mybir.AluOpType
ACTF = mybir.ActivationFunctionType

C, CC, H, W = 256, 64, 128, 128
B = 4
HO, WO = 32, 64           # per-core output dims
NPOS = HO * WO            # 2048
K5 = 5
GH, GW = 34, 66           # subgrid dims (rows, cols)
GSZ = GH * GW             # 2244 flat
# slabs: (x_row_lo, x_row_hi, out_row_lo, n_out_rows); a block of n
# output rows starting at ho reads x subgrid rows [ho, ho+n+2)
SLABS = [(0, 18, 0, 16), (18, 34, 16, 16)]
TPS = 8                   # 128-pos tiles per slab


def build_nc():
    nc = bacc.Bacc("TRN2", target_bir_lowering=False, debug=False)

    # inputs
    xq = nc.dram_tensor("xq", [C, 4, GH, GW], F16, kind="ExternalInput")
    wc = nc.dram_tensor("wc", [128, 18 * 41], F16, kind="ExternalInput")
    ident = nc.dram_tensor("ident", [128, 128], F32, kind="ExternalInput")
    mscr = nc.dram_tensor("mscr", [25, NPOS], F16)
    y = nc.dram_tensor("y", [C, HO, WO], F16, kind="ExternalOutput")

    with tile.TileContext(nc) as tc:
        with (
            tc.tile_pool(name="big", bufs=1) as bigpool,
            tc.tile_pool(name="work", bufs=3) as workpool,
            tc.tile_pool(name="tmp", bufs=6) as tmppool,
            tc.tile_pool(name="mbp", bufs=8) as mbpool,
            tc.tile_pool(name="ps2", bufs=3, space="PSUM") as ps2,
            tc.tile_pool(name="ps3", bufs=1, space="PSUM") as ps3,
            tc.tile_pool(name="ps4", bufs=1, space="PSUM") as ps4,
            tc.tile_pool(name="ps5", bufs=1, space="PSUM") as ps5,
        ):
            # ---- persistent tiles ----
            xboth = bigpool.tile([128, 2, 4, GH, GW], F16, tag="xboth")
            wcs = bigpool.tile([128, 18 * 41], F16, tag="wc")
            ids = bigpool.tile([128, 128], F32, tag="ident")
            logits = bigpool.tile([41, NPOS], F32, tag="logits")
            mcm = bigpool.tile([25, NPOS], F16, tag="mcm")
            accboth = bigpool.tile([128, 2, HO, WO], F16, tag="acc")
            ewarm = bigpool.tile([1, 1], F32, tag="ewarm")

            nc.sync.dma_start(out=wcs[:], in_=wc[:])
            nc.sync.dma_start(out=ids[:], in_=ident[:])
            # x DMAs: slab-0 rows (0:10) first for all subgrids so the
            # first conv block starts early, then the bulk. DMA queue
            # issue costs are ~650ns flat per dma_start.
            # g3 first: the conv taps consume subgrids in order 3,2,1,0
            for g in (3, 2, 1, 0):
                nc.sync.dma_start(out=xboth[:, 0, g, 0:18],
                                  in_=xq[0:128, g, 0:18])
                nc.sync.dma_start(out=xboth[:, 1, g, 0:18],
                                  in_=xq[128:256, g, 0:18])
            for g in (3, 2, 1, 0):
                nc.sync.dma_start(out=xboth[:, 0, g, 18:34],
                                  in_=xq[0:128, g, 18:34])
                nc.sync.dma_start(out=xboth[:, 1, g, 18:34],
                                  in_=xq[128:256, g, 18:34])
            # pre-warm the exp activation table during the x DMAs
            nc.scalar.activation(ewarm[:], wcs[0:1, 0:1], ACTF.Exp)
            # pre-warm the PE HAM clock gate (K=4/8 -> 8/8 takes ~4us of
            # sustained matmuls) while the x DMAs are still in flight
            pwrm = ps5.tile([41, 512], F32, tag="ps5")
            for _ in range(10):
                nc.tensor.matmul(pwrm[:], wcs[:, 0:41],
                                 wcs[:, 128:640], start=True, stop=True)

            yf = y.rearrange("c h w -> c (h w)")

            mbs = {}

            # composed conv weights: logits = (w_enc|w_kenc) ∘ w_comp
            # applied directly to x; 9 3x3-taps x 2 channel chips, each a
            # [contract 128 -> 41] matmul accumulating in PSUM
            TAPS9 = [(3, 0, 0), (3, 0, 1), (3, 1, 0), (3, 1, 1),
                     (2, 0, 1), (2, 1, 1),
                     (1, 1, 0), (1, 1, 1),
                     (0, 1, 1)]  # (g, dh, dw), grouped by x-load order

            def emit_conv_block(hob):
                # fused conv -> logits for 8 output rows at hob
                lgp = ps2.tile([41, 512], F32, tag="ps2")
                nmm = 0
                for j, (g, dh, dw) in enumerate(TAPS9):
                    for ch in range(2):
                        rhs = xboth[:, ch, g, hob + dh: hob + dh + 8,
                                    dw: dw + 64]
                        o = 41 * (2 * j + ch)
                        nc.tensor.matmul(lgp[:], wcs[:, o: o + 41],
                                         rhs, start=(nmm == 0),
                                         stop=(nmm == 17))
                        nmm += 1
                nc.scalar.activation(
                    logits[:, 64 * hob: 64 * hob + 512],
                    lgp[:], ACTF.Copy)

            mskns = {}

            def emit_slab_front(si):
                ra, rb, ho, nr = SLABS[si]
                pos0 = 64 * ho
                nps = 64 * nr
                nt = nps // 128
                # ---- transpose logits -> pos-major (one PSUM tile) ----
                lgTp = ps3.tile([128, TPS, 41], F32, tag="ps3")
                for tt in range(nt):
                    t = pos0 // 128 + tt
                    nc.tensor.transpose(lgTp[:, tt, :],
                                        logits[:, 128 * t: 128 * (t + 1)],
                                        ids[0:41, 0:41])
                lgT = workpool.tile([128, TPS, 41], F32, tag="lgT")
                nc.scalar.activation(lgT[:, 0:nt], lgTp[:, 0:nt], ACTF.Copy)

                # ---- mask pipeline on GpSimd (exp fp32-safe w/o max) ----
                p8 = workpool.tile([128, TPS, 8], F32, tag="p8")
                nc.gpsimd.tensor_tensor(p8[:, 0:nt], lgT[:, 0:nt, 25:33],
                                        lgT[:, 0:nt, 33:41], OP.mult)
                p4 = workpool.tile([128, TPS, 4], F32, tag="p4")
                nc.gpsimd.tensor_tensor(p4[:, 0:nt], p8[:, 0:nt, 0:4],
                                        p8[:, 0:nt, 4:8], OP.mult)
                p2 = workpool.tile([128, TPS, 2], F32, tag="p2")
                nc.gpsimd.tensor_tensor(p2[:, 0:nt], p4[:, 0:nt, 0:2],
                                        p4[:, 0:nt, 2:4], OP.mult)
                i0 = workpool.tile([128, TPS], F32, tag="i0")
                nc.gpsimd.tensor_tensor(i0[:, 0:nt], p2[:, 0:nt, 0],
                                        p2[:, 0:nt, 1], OP.mult)
                ic = workpool.tile([128, TPS], F32, tag="ic")
                nc.gpsimd.tensor_scalar(ic[:, 0:nt], i0[:, 0:nt], 10.0,
                                        -10.0, OP.min, OP.max)

                mskl = workpool.tile([128, TPS, 25], F32, tag="mskl")
                nc.gpsimd.tensor_tensor(
                    mskl[:, 0:nt], lgT[:, 0:nt, 0:25],
                    ic[:, 0:nt].to_broadcast([128, nt, 25]), OP.mult)
                mexp = workpool.tile([128, TPS, 25], F32, tag="mexp")
                nc.scalar.activation(mexp[:, 0:nt], mskl[:, 0:nt], ACTF.Exp)
                msum = workpool.tile([128, TPS], F32, tag="msum")
                nc.vector.tensor_reduce(msum[:, 0:nt], mexp[:, 0:nt], AX.X,
                                        OP.add)
                mrec = workpool.tile([128, TPS], F32, tag="mrec")
                nc.vector.reciprocal(mrec[:, 0:nt], msum[:, 0:nt])
                mskn = workpool.tile([128, TPS, 25], F32, tag="mskn")
                nc.gpsimd.tensor_tensor(
                    mskn[:, 0:nt], mexp[:, 0:nt],
                    mrec[:, 0:nt].to_broadcast([128, nt, 25]), OP.mult)
                mskns[si] = mskn

            def emit_slab_back(si):
                ra, rb, ho, nr = SLABS[si]
                pos0 = 64 * ho
                nps = 64 * nr
                nt = nps // 128
                mskn = mskns[si]
                # ---- transpose mask back to channel-major (fp16) ----
                mcp = ps4.tile([25, TPS, 128], F32, tag="ps4")
                for tt in range(nt):
                    nc.tensor.transpose(mcp[:, tt, :], mskn[:, tt, :],
                                        ids[:])
                nc.scalar.activation(
                    mcm[:, pos0: pos0 + nps],
                    mcp[:, 0:nt].rearrange("k t p -> k (t p)"), ACTF.Copy)
                # bounce this slab's masks to DRAM right away (single
                # writer of this mscr region -> sound broadcast deps)
                nc.sync.dma_start(out=mscr[:, pos0: pos0 + nps],
                                  in_=mcm[:, pos0: pos0 + nps])



            def emit_reassembly(row0, nrows, cname):
                # rows [row0, row0+nrows): 3-dim FD=nrows*64 APs per
                # (tap, chip); chip chains alternate so consecutive DVE
                # ops are independent. mb broadcasts are throttled by
                # the 8-slot mb pool. One bounce per chunk region keeps
                # a single mscr writer per broadcast read (sound deps).
                # broadcasts ride the (now idle) PE queue, one DMA per
                # slab-half so each read has a single mscr writer
                pos0, npos = 64 * row0, 64 * nrows
                accv = [accboth[:, ch, row0: row0 + nrows, :]
                        for ch in range(2)]
                for k in range(K5 * K5):
                    mb = mbpool.tile([128, npos], F16, tag=f"mb{cname}")
                    for si, (sa, sb, so, sn) in enumerate(SLABS):
                        p0, pn = 64 * so, 64 * sn
                        eng = nc.gpsimd if si == 0 else nc.sync
                        eng.dma_start(
                            out=mb[:, p0 - pos0: p0 - pos0 + pn],
                            in_=mscr[k: k + 1,
                                     p0: p0 + pn].to_broadcast([128, pn]))
                    mbs[k] = mb.rearrange("p (h w) -> p h w", h=nrows)
                for k in range(K5 * K5):
                    ky, kx = k // K5, k % K5
                    g = 2 * (ky % 2) + (kx % 2)
                    ts = []
                    for ch in range(2):
                        xsrc = xboth[:, ch, g,
                                     row0 + ky // 2: row0 + ky // 2 + nrows,
                                     kx // 2: kx // 2 + 64]
                        if k == 0:
                            nc.vector.tensor_tensor(accv[ch], xsrc,
                                                    mbs[k][:], OP.mult)
                        else:
                            t = tmppool.tile([128, nrows, 64], F16,
                                             tag=f"tp{cname}")
                            nc.vector.tensor_tensor(t[:], xsrc, mbs[k][:],
                                                    OP.mult)
                            ts.append(t)
                    for ch, t in enumerate(ts):
                        nc.vector.tensor_tensor(accv[ch], accv[ch], t[:],
                                                OP.add)

                # ---- store this chunk (fp16, host upcasts) ----
                for ch in range(2):
                    nc.scalar.dma_start(
                        out=yf[128 * ch: 128 * (ch + 1),
                               pos0: pos0 + npos],
                        in_=accboth[:, ch, row0: row0 + nrows, :].rearrange(
                            "p h w -> p (h w)"))

            # ---- pipeline: all conv blocks back-to-back (keeps the PE
            # HAM warm), then both mask-slab fronts (so the two softmax
            # chains pipeline), then the mask transposes, then one full
            # reassembly ----
            for hob in range(0, HO, 8):
                emit_conv_block(hob)
            emit_slab_front(0)
            emit_slab_front(1)
            emit_slab_back(0)
            emit_slab_back(1)
            emit_reassembly(0, HO, "a")

    nc.finalize()
    return nc


def make_core_inputs(x, w_comp, b_comp, w_enc, b_enc, w_kenc, b_kenc):
    """Full inputs -> list of 8 per-core input dicts."""
    x = np.asarray(x)
    w1 = np.asarray(w_comp).reshape(CC, C).astype(np.float64)  # [64, 256]
    we = np.asarray(w_enc)    # [25, 64, 3, 3]
    wk = np.asarray(w_kenc)   # [16, 64, 3, 3]
    w41 = np.concatenate([we, wk], axis=0).astype(np.float64)  # [41,64,3,3]
    # compose: logits-conv applied directly to x (all biases are zero)
    wq = np.einsum('kcyx,cd->kdyx', w41, w1)   # [41, 256, 3, 3]

    # stationaries per (3x3 tap, chip): wc[p, (2j+ch)*41 + k]
    # tap order must match kernel TAPS9 (grouped by subgrid 3,2,1,0)
    TAPS9_YX = [(0, 0), (0, 2), (2, 0), (2, 2),
                (0, 1), (2, 1), (1, 0), (1, 2), (1, 1)]
    wch = np.zeros((128, 18, 41), np.float64)
    for j, (ty, tx) in enumerate(TAPS9_YX):
        for ch in range(2):
            wch[:, 2 * j + ch] = wq[:, 128 * ch: 128 * (ch + 1), ty, tx].T
    wch = wch.reshape(128, 18 * 41).astype(np.float16)
    ident = np.eye(128, dtype=np.float32)

    maps = []
    for core in range(8):
        b, h = core // 2, core % 2
        start = 64 * h
        xpc = np.zeros((C, 68, 132), np.float32)
        lo, hi = start - 2, start + 66
        clo, chi = max(lo, 0), min(hi, H)
        xpc[:, clo - lo: clo - lo + (chi - clo), 2:130] = x[b, :, clo:chi, :]
        # de-interleave: g = 2*(row%2) + (col%2)
        xqc = np.empty((C, 4, GH, GW), np.float16)
        xqc[:, 0] = xpc[:, 0::2, 0::2]
        xqc[:, 1] = xpc[:, 0::2, 1::2]
        xqc[:, 2] = xpc[:, 1::2, 0::2]
        xqc[:, 3] = xpc[:, 1::2, 1::2]
        maps.append({
            "xq": xqc,
            "wc": wch,
            "ident": ident,
        })
    return maps


def assemble_output(results):
    out = np.zeros((B, C, 64, 64), np.float32)
    for core in range(8):
        b, h = core // 2, core % 2
        out[b, :, 32 * h: 32 * (h + 1), :] = results[core]["y"].astype(
            np.float32)
    return out


_NC_CACHE = []


def kernel(**inputs):
    import numpy as _np
    from concourse.bass_utils import run_bass_kernel_spmd

    maps = make_core_inputs(
        inputs["x"], inputs["w_comp"], inputs["b_comp"], inputs["w_enc"],
        inputs["b_enc"], inputs["w_kenc"], inputs["b_kenc"])
    if not _NC_CACHE:
        _NC_CACHE.append(build_nc())
    res = run_bass_kernel_spmd(_NC_CACHE[0], maps, list(range(8)))
    out = assemble_output(res.results)
    return out.astype(_np.float32)


# revision 57
# speedup vs baseline: 1.0211x; 1.0211x over previous
"""CARAFE-downsample (K=5, stride=2) Trainium2 kernel, 8-core SPMD.

Key ideas:
- Host de-interleaves x into 4 parity subgrids (fp16) so every
  stride-2 access (conv3x3 taps + all 25 reassembly taps) becomes a
  contiguous slice.
- Both 128-channel chips live in ONE SBUF tile (chip dim inside the
  free dims), so each reassembly mult/add is a single FD=2048 DVE op
  instead of two FD=1024 ops: 49 ops/chunk instead of 98, amortizing
  the ~160ns/op DVE overhead.
- conv+softmax mask production is split into 4 row-slabs (8 output
  rows each) and issued ahead of reassembly (A0 A1 R0 A2 A3 R1), so
  the serial head is only two slabs deep and chunk-1 masks are
  produced while chunk-0 reassembly runs.
- Masks bounce through DRAM per half-chunk and are broadcast to 128
  partitions by replicating DMAs; mb pool slots auto-throttle the
  prefetch of chunk-1 masks during chunk-0 reassembly.
- Reassembly adds use two independent accumulator chains to avoid
  back-to-back RAW stalls on the DVE.
- Output stays fp16 and is DMA'd straight from the accumulator
  (host upcasts); mask-softmax path stays fp32 (the 16-way channel
  product amplifies logit errors).
Sharding: core = batch*2 + H-half; zero-padded 2-row/2-col halo.
"""

import numpy as np

import concourse.bacc as bacc
import concourse.mybir as mybir
import concourse.tile as tile

F32 = mybir.dt.float32
F16 = mybir.dt.float16
AX = mybir.AxisListType
OP = mybir.AluOpType
ACTF = mybir.ActivationFunctionType

C, CC, H, W = 256, 64, 128, 128
B = 4
HO, WO = 32, 64           # per-core output dims
NPOS = HO * WO            # 2048
K5 = 5
GH, GW = 34, 66           # subgrid dims (rows, cols)
GSZ = GH * GW             # 2244 flat
# slabs: (x_row_lo, x_row_hi, out_row_lo, n_out_rows); a block of n
# output rows starting at ho reads x subgrid rows [ho, ho+n+2)
SLABS = [(0, 18, 0, 16), (18, 34, 16, 16)]
TPS = 8                   # 128-pos tiles per slab


def build_nc():
    nc = bacc.Bacc("TRN2", target_bir_lowering=False, debug=False)

    # inputs
    xq = nc.dram_tensor("xq", [C, 4, GH, GW], F16, kind="ExternalInput")
    wc = nc.dram_tensor("wc", [128, 18 * 41], F16, kind="ExternalInput")
    ident = nc.dram_tensor("ident", [128, 128], F32, kind="ExternalInput")
    mscr = nc.dram_tensor("mscr", [25, NPOS], F16)
    y = nc.dram_tensor("y", [C, HO, WO], F16, kind="ExternalOutput")

    with tile.TileContext(nc) as tc:
        with (
            tc.tile_pool(name="big", bufs=1) as bigpool,
            tc.tile_pool(name="work", bufs=3) as workpool,
            tc.tile_pool(name="tmp", bufs=6) as tmppool,
            tc.tile_pool(name="mbp", bufs=8) as mbpool,
            tc.tile_pool(name="ps2", bufs=3, space="PSUM") as ps2,
            tc.tile_pool(name="ps3", bufs=1, space="PSUM") as ps3,
            tc.tile_pool(name="ps4", bufs=1, space="PSUM") as ps4,
            tc.tile_pool(name="ps5", bufs=1, space="PSUM") as ps5,
        ):
            # ---- persistent tiles ----
            xboth = bigpool.tile([128, 2, 4, GH, GW], F16, tag="xboth")
            wcs = bigpool.tile([128, 18 * 41], F16, tag="wc")
            ids = bigpool.tile([128, 128], F32, tag="ident")
            logits = bigpool.tile([41, NPOS], F32, tag="logits")
            mcm = bigpool.tile([25, NPOS], F16, tag="mcm")
            accboth = bigpool.tile([128, 2, HO, WO], F16, tag="acc")
            ewarm = bigpool.tile([1, 1], F32, tag="ewarm")

            nc.sync.dma_start(out=wcs[:], in_=wc[:])
            nc.sync.dma_start(out=ids[:], in_=ident[:])
            # x DMAs: slab-0 rows (0:10) first for all subgrids so the
            # first conv block starts early, then the bulk. DMA queue
            # issue costs are ~650ns flat per dma_start.
            # g3 first: the conv taps consume subgrids in order 3,2,1,0
            for g in (3, 2, 1, 0):
                nc.sync.dma_start(out=xboth[:, 0, g, 0:18],
                                  in_=xq[0:128, g, 0:18])
                nc.sync.dma_start(out=xboth[:, 1, g, 0:18],
                                  in_=xq[128:256, g, 0:18])
            for g in (3, 2, 1, 0):
                nc.sync.dma_start(out=xboth[:, 0, g, 18:34],
                                  in_=xq[0:128, g, 18:34])
                nc.sync.dma_start(out=xboth[:, 1, g, 18:34],
                                  in_=xq[128:256, g, 18:34])
            # pre-warm the exp activation table during the x DMAs
            nc.scalar.activation(ewarm[:], wcs[0:1, 0:1], ACTF.Exp)
            # pre-warm the PE HAM clock gate (K=4/8 -> 8/8 takes ~4us of
            # sustained matmuls) while the x DMAs are still in flight
            pwrm = ps5.tile([41, 512], F32, tag="ps5")
            for _ in range(10):
                nc.tensor.matmul(pwrm[:], wcs[:, 0:41],
                                 wcs[:, 128:640], start=True, stop=True)

            yf = y.rearrange("c h w -> c (h w)")

            mbs = {}

            # composed conv weights: logits = (w_enc|w_kenc) ∘ w_comp
            # applied directly to x; 9 3x3-taps x 2 channel chips, each a
            # [contract 128 -> 41] matmul accumulating in PSUM
            TAPS9 = [(3, 0, 0), (3, 0, 1), (3, 1, 0), (3, 1, 1),
                     (2, 0, 1), (2, 1, 1),
                     (1, 1, 0), (1, 1, 1),
                     (0, 1, 1)]  # (g, dh, dw), grouped by x-load order

            def emit_conv_block(hob):
                # fused conv -> logits for 8 output rows at hob
                lgp = ps2.tile([41, 512], F32, tag="ps2")
                nmm = 0
                for j, (g, dh, dw) in enumerate(TAPS9):
                    for ch in range(2):
                        rhs = xboth[:, ch, g, hob + dh: hob + dh + 8,
                                    dw: dw + 64]
                        o = 41 * (2 * j + ch)
                        nc.tensor.matmul(lgp[:], wcs[:, o: o + 41],
                                         rhs, start=(nmm == 0),
                                         stop=(nmm == 17))
                        nmm += 1
                nc.scalar.activation(
                    logits[:, 64 * hob: 64 * hob + 512],
                    lgp[:], ACTF.Copy)

            mskns = {}

            def emit_slab_front(si):
                ra, rb, ho, nr = SLABS[si]
                pos0 = 64 * ho
                nps = 64 * nr
                nt = nps // 128
                # ---- transpose logits -> pos-major (one PSUM tile) ----
                lgTp = ps3.tile([128, TPS, 41], F32, tag="ps3")
                for tt in range(nt):
                    t = pos0 // 128 + tt
                    nc.tensor.transpose(lgTp[:, tt, :],
                                        logits[:, 128 * t: 128 * (t + 1)],
                                        ids[0:41, 0:41])
                lgT = workpool.tile([128, TPS, 41], F32, tag="lgT")
                nc.scalar.activation(lgT[:, 0:nt], lgTp[:, 0:nt], ACTF.Copy)

                # ---- mask pipeline on GpSimd (exp fp32-safe w/o max) ----
                p8 = workpool.tile([128, TPS, 8], F32, tag="p8")
                nc.gpsimd.tensor_tensor(p8[:, 0:nt], lgT[:, 0:nt, 25:33],
                                        lgT[:, 0:nt, 33:41], OP.mult)
                p4 = workpool.tile([128, TPS, 4], F32, tag="p4")
                nc.gpsimd.tensor_tensor(p4[:, 0:nt], p8[:, 0:nt, 0:4],
                                        p8[:, 0:nt, 4:8], OP.mult)
                p2 = workpool.tile([128, TPS, 2], F32, tag="p2")
                nc.gpsimd.tensor_tensor(p2[:, 0:nt], p4[:, 0:nt, 0:2],
                                        p4[:, 0:nt, 2:4], OP.mult)
                i0 = workpool.tile([128, TPS], F32, tag="i0")
                nc.gpsimd.tensor_tensor(i0[:, 0:nt], p2[:, 0:nt, 0],
                                        p2[:, 0:nt, 1], OP.mult)
                ic = workpool.tile([128, TPS], F32, tag="ic")
                nc.gpsimd.tensor_scalar(ic[:, 0:nt], i0[:, 0:nt], 10.0,
                                        -10.0, OP.min, OP.max)

                mskl = workpool.tile([128, TPS, 25], F32, tag="mskl")
                nc.gpsimd.tensor_tensor(
                    mskl[:, 0:nt], lgT[:, 0:nt, 0:25],
                    ic[:, 0:nt].to_broadcast([128, nt, 25]), OP.mult)
                mexp = workpool.tile([128, TPS, 25], F32, tag="mexp")
                nc.scalar.activation(mexp[:, 0:nt], mskl[:, 0:nt], ACTF.Exp)
                msum = workpool.tile([128, TPS], F32, tag="msum")
                nc.vector.tensor_reduce(msum[:, 0:nt], mexp[:, 0:nt], AX.X,
                                        OP.add)
                mrec = workpool.tile([128, TPS], F32, tag="mrec")
                nc.vector.reciprocal(mrec[:, 0:nt], msum[:, 0:nt])
                mskn = workpool.tile([128, TPS, 25], F32, tag="mskn")
                nc.gpsimd.tensor_tensor(
                    mskn[:, 0:nt], mexp[:, 0:nt],
                    mrec[:, 0:nt].to_broadcast([128, nt, 25]), OP.mult)
                mskns[si] = mskn

            def emit_slab_back(si):
                ra, rb, ho, nr = SLABS[si]
                pos0 = 64 * ho
                nps = 64 * nr
                nt = nps // 128
                mskn = mskns[si]
                # ---- transpose mask back to channel-major (fp16) ----
                mcp = ps4.tile([25, TPS, 128], F32, tag="ps4")
                for tt in range(nt):
                    nc.tensor.transpose(mcp[:, tt, :], mskn[:, tt, :],
                                        ids[:])
                nc.scalar.activation(
                    mcm[:, pos0: pos0 + nps],
                    mcp[:, 0:nt].rearrange("k t p -> k (t p)"), ACTF.Copy)
                # bounce this slab's masks to DRAM right away (single
                # writer of this mscr region -> sound broadcast deps)
                nc.sync.dma_start(out=mscr[:, pos0: pos0 + nps],
                                  in_=mcm[:, pos0: pos0 + nps])



            def emit_reassembly(row0, nrows, cname):
                # rows [row0, row0+nrows): 3-dim FD=nrows*64 APs per
                # (tap, chip); chip chains alternate so consecutive DVE
                # ops are independent. mb broadcasts are throttled by
                # the 8-slot mb pool. One bounce per chunk region keeps
                # a single mscr writer per broadcast read (sound deps).
                # broadcasts ride the (now idle) PE queue, one DMA per
                # slab-half so each read has a single mscr writer
                pos0, npos = 64 * row0, 64 * nrows
                accv = [accboth[:, ch, row0: row0 + nrows, :]
                        for ch in range(2)]
                for k in range(K5 * K5):
                    mb = mbpool.tile([128, npos], F16, tag=f"mb{cname}")
                    for si, (sa, sb, so, sn) in enumerate(SLABS):
                        p0, pn = 64 * so, 64 * sn
                        nc.sync.dma_start(
                            out=mb[:, p0 - pos0: p0 - pos0 + pn],
                            in_=mscr[k: k + 1,
                                     p0: p0 + pn].to_broadcast([128, pn]))
                    mbs[k] = mb.rearrange("p (h w) -> p h w", h=nrows)
                for k in range(K5 * K5):
                    ky, kx = k // K5, k % K5
                    g = 2 * (ky % 2) + (kx % 2)
                    ts = []
                    for ch in range(2):
                        xsrc = xboth[:, ch, g,
                                     row0 + ky // 2: row0 + ky // 2 + nrows,
                                     kx // 2: kx // 2 + 64]
                        if k == 0:
                            nc.vector.tensor_tensor(accv[ch], xsrc,
                                                    mbs[k][:], OP.mult)
                        else:
                            t = tmppool.tile([128, nrows, 64], F16,
                                             tag=f"tp{cname}")
                            nc.vector.tensor_tensor(t[:], xsrc, mbs[k][:],
                                                    OP.mult)
                            ts.append(t)
                    for ch, t in enumerate(ts):
                        nc.vector.tensor_tensor(accv[ch], accv[ch], t[:],
                                                OP.add)

                # ---- store this chunk (fp16, host upcasts) ----
                for ch in range(2):
                    nc.scalar.dma_start(
                        out=yf[128 * ch: 128 * (ch + 1),
                               pos0: pos0 + npos],
                        in_=accboth[:, ch, row0: row0 + nrows, :].rearrange(
                            "p h w -> p (h w)"))

            # ---- pipeline: all conv blocks back-to-back (keeps the PE
            # HAM warm), then both mask-slab fronts (so the two softmax
            # chains pipeline), then the mask transposes, then one full
            # reassembly ----
            for hob in range(0, HO, 8):
                emit_conv_block(hob)
            emit_slab_front(0)
            emit_slab_front(1)
            emit_slab_back(0)
            emit_slab_back(1)
            emit_reassembly(0, HO, "a")

    nc.finalize()
    return nc


def make_core_inputs(x, w_comp, b_comp, w_enc, b_enc, w_kenc, b_kenc):
    """Full inputs -> list of 8 per-core input dicts."""
    x = np.asarray(x)
    w1 = np.asarray(w_comp).reshape(CC, C).astype(np.float64)  # [64, 256]
    we = np.asarray(w_enc)    # [25, 64, 3, 3]
    wk = np.asarray(w_kenc)   # [16, 64, 3, 3]
    w41 = np.concatenate([we, wk], axis=0).astype(np.float64)  # [41,64,3,3]
    # compose: logits-conv applied directly to x (all biases are zero)
    wq = np.einsum('kcyx,cd->kdyx', w41, w1)   # [41, 256, 3, 3]

    # stationaries per (3x3 tap, chip): wc[p, (2j+ch)*41 + k]
    # tap order must match kernel TAPS9 (grouped by subgrid 3,2,1,0)
    TAPS9_YX = [(0, 0), (0, 2), (2, 0), (2, 2),
                (0, 1), (2, 1), (1, 0), (1, 2), (1, 1)]
    wch = np.zeros((128, 18, 41), np.float64)
    for j, (ty, tx) in enumerate(TAPS9_YX):
        for ch in range(2):
            wch[:, 2 * j + ch] = wq[:, 128 * ch: 128 * (ch + 1), ty, tx].T
    wch = wch.reshape(128, 18 * 41).astype(np.float16)
    ident = np.eye(128, dtype=np.float32)

    maps = []
    for core in range(8):
        b, h = core // 2, core % 2
        start = 64 * h
        xpc = np.zeros((C, 68, 132), np.float32)
        lo, hi = start - 2, start + 66
        clo, chi = max(lo, 0), min(hi, H)
        xpc[:, clo - lo: clo - lo + (chi - clo), 2:130] = x[b, :, clo:chi, :]
        # de-interleave: g = 2*(row%2) + (col%2)
        xqc = np.empty((C, 4, GH, GW), np.float16)
        xqc[:, 0] = xpc[:, 0::2, 0::2]
        xqc[:, 1] = xpc[:, 0::2, 1::2]
        xqc[:, 2] = xpc[:, 1::2, 0::2]
        xqc[:, 3] = xpc[:, 1::2, 1::2]
        maps.append({
            "xq": xqc,
            "wc": wch,
            "ident": ident,
        })
    return maps


def assemble_output(results):
    out = np.zeros((B, C, 64, 64), np.float32)
    for core in range(8):
        b, h = core // 2, core % 2
        out[b, :, 32 * h: 32 * (h + 1), :] = results[core]["y"].astype(
            np.float32)
    return out


_NC_CACHE = []


def kernel(**inputs):
    import numpy as _np
    from concourse.bass_utils import run_bass_kernel_spmd

    maps = make_core_inputs(
        inputs["x"], inputs["w_comp"], inputs["b_comp"], inputs["w_enc"],
        inputs["b_enc"], inputs["w_kenc"], inputs["b_kenc"])
    if not _NC_CACHE:
        _NC_CACHE.append(build_nc())
    res = run_bass_kernel_spmd(_NC_CACHE[0], maps, list(range(8)))
    out = assemble_output(res.results)
    return out.astype(_np.float32)


# revision 59
# speedup vs baseline: 1.0257x; 1.0045x over previous
"""CARAFE-downsample (K=5, stride=2) Trainium2 kernel, 8-core SPMD.

Key ideas:
- Host de-interleaves x into 4 parity subgrids (fp16) so every
  stride-2 access (conv3x3 taps + all 25 reassembly taps) becomes a
  contiguous slice.
- Both 128-channel chips live in ONE SBUF tile (chip dim inside the
  free dims), so each reassembly mult/add is a single FD=2048 DVE op
  instead of two FD=1024 ops: 49 ops/chunk instead of 98, amortizing
  the ~160ns/op DVE overhead.
- conv+softmax mask production is split into 4 row-slabs (8 output
  rows each) and issued ahead of reassembly (A0 A1 R0 A2 A3 R1), so
  the serial head is only two slabs deep and chunk-1 masks are
  produced while chunk-0 reassembly runs.
- Masks bounce through DRAM per half-chunk and are broadcast to 128
  partitions by replicating DMAs; mb pool slots auto-throttle the
  prefetch of chunk-1 masks during chunk-0 reassembly.
- Reassembly adds use two independent accumulator chains to avoid
  back-to-back RAW stalls on the DVE.
- Output stays fp16 and is DMA'd straight from the accumulator
  (host upcasts); mask-softmax path stays fp32 (the 16-way channel
  product amplifies logit errors).
Sharding: core = batch*2 + H-half; zero-padded 2-row/2-col halo.
"""

import numpy as np

import concourse.bacc as bacc
import concourse.mybir as mybir
import concourse.tile as tile

F32 = mybir.dt.float32
F16 = mybir.dt.float16
AX = mybir.AxisListType
OP = mybir.AluOpType
ACTF = mybir.ActivationFunctionType

C, CC, H, W = 256, 64, 128, 128
B = 4
HO, WO = 32, 64           # per-core output dims
NPOS = HO * WO            # 2048
K5 = 5
GH, GW = 34, 66           # subgrid dims (rows, cols)
GSZ = GH * GW             # 2244 flat
# slabs: (x_row_lo, x_row_hi, out_row_lo, n_out_rows); a block of n
# output rows starting at ho reads x subgrid rows [ho, ho+n+2)
SLABS = [(0, 18, 0, 16), (18, 34, 16, 16)]
TPS = 8                   # 128-pos tiles per slab


def build_nc():
    nc = bacc.Bacc("TRN2", target_bir_lowering=False, debug=False)

    # inputs
    xq = nc.dram_tensor("xq", [C, 4, GH, GW], F16, kind="ExternalInput")
    wc = nc.dram_tensor("wc", [128, 18 * 41], F16, kind="ExternalInput")
    ident = nc.dram_tensor("ident", [128, 128], F32, kind="ExternalInput")
    mscr = nc.dram_tensor("mscr", [25, NPOS], F16)
    y = nc.dram_tensor("y", [C, HO, WO], F16, kind="ExternalOutput")

    with tile.TileContext(nc) as tc:
        with (
            tc.tile_pool(name="big", bufs=1) as bigpool,
            tc.tile_pool(name="work", bufs=3) as workpool,
            tc.tile_pool(name="tmp", bufs=6) as tmppool,
            tc.tile_pool(name="mbp", bufs=8) as mbpool,
            tc.tile_pool(name="ps2", bufs=3, space="PSUM") as ps2,
            tc.tile_pool(name="ps3", bufs=1, space="PSUM") as ps3,
            tc.tile_pool(name="ps4", bufs=1, space="PSUM") as ps4,
            tc.tile_pool(name="ps5", bufs=1, space="PSUM") as ps5,
        ):
            # ---- persistent tiles ----
            xboth = bigpool.tile([128, 2, 4, GH, GW], F16, tag="xboth")
            wcs = bigpool.tile([128, 18 * 41], F16, tag="wc")
            ids = bigpool.tile([128, 128], F32, tag="ident")
            logits = bigpool.tile([41, NPOS], F32, tag="logits")
            mcm = bigpool.tile([25, NPOS], F16, tag="mcm")
            accboth = bigpool.tile([128, 2, HO, WO], F16, tag="acc")
            ewarm = bigpool.tile([1, 1], F32, tag="ewarm")

            nc.sync.dma_start(out=wcs[:], in_=wc[:])
            nc.sync.dma_start(out=ids[:], in_=ident[:])
            # x DMAs: slab-0 rows (0:10) first for all subgrids so the
            # first conv block starts early, then the bulk. DMA queue
            # issue costs are ~650ns flat per dma_start.
            # g3 first: the conv taps consume subgrids in order 3,2,1,0
            for g in (3, 2, 1, 0):
                nc.sync.dma_start(out=xboth[:, 0, g, 0:18],
                                  in_=xq[0:128, g, 0:18])
                nc.sync.dma_start(out=xboth[:, 1, g, 0:18],
                                  in_=xq[128:256, g, 0:18])
            for g in (3, 2, 1, 0):
                nc.sync.dma_start(out=xboth[:, 0, g, 18:34],
                                  in_=xq[0:128, g, 18:34])
                nc.sync.dma_start(out=xboth[:, 1, g, 18:34],
                                  in_=xq[128:256, g, 18:34])
            # pre-warm the exp activation table during the x DMAs
            nc.scalar.activation(ewarm[:], wcs[0:1, 0:1], ACTF.Exp)
            # pre-warm the PE HAM clock gate (K=4/8 -> 8/8 takes ~4us of
            # sustained matmuls) while the x DMAs are still in flight
            pwrm = ps5.tile([41, 512], F32, tag="ps5")
            for _ in range(10):
                nc.tensor.matmul(pwrm[:], wcs[:, 0:41],
                                 wcs[:, 128:640], start=True, stop=True)

            yf = y.rearrange("c h w -> c (h w)")

            mbs = {}

            # composed conv weights: logits = (w_enc|w_kenc) ∘ w_comp
            # applied directly to x; 9 3x3-taps x 2 channel chips, each a
            # [contract 128 -> 41] matmul accumulating in PSUM
            TAPS9 = [(3, 0, 0), (3, 0, 1), (3, 1, 0), (3, 1, 1),
                     (2, 0, 1), (2, 1, 1),
                     (1, 1, 0), (1, 1, 1),
                     (0, 1, 1)]  # (g, dh, dw), grouped by x-load order

            def emit_conv_block(hob):
                # fused conv -> logits for 8 output rows at hob
                lgp = ps2.tile([41, 512], F32, tag="ps2")
                nmm = 0
                for j, (g, dh, dw) in enumerate(TAPS9):
                    for ch in range(2):
                        rhs = xboth[:, ch, g, hob + dh: hob + dh + 8,
                                    dw: dw + 64]
                        o = 41 * (2 * j + ch)
                        nc.tensor.matmul(lgp[:], wcs[:, o: o + 41],
                                         rhs, start=(nmm == 0),
                                         stop=(nmm == 17))
                        nmm += 1
                nc.scalar.activation(
                    logits[:, 64 * hob: 64 * hob + 512],
                    lgp[:], ACTF.Copy)

            mskns = {}

            def emit_slab_front(si):
                ra, rb, ho, nr = SLABS[si]
                pos0 = 64 * ho
                nps = 64 * nr
                nt = nps // 128
                # ---- transpose logits -> pos-major (one PSUM tile) ----
                lgTp = ps3.tile([128, TPS, 41], F32, tag="ps3")
                for tt in range(nt):
                    t = pos0 // 128 + tt
                    nc.tensor.transpose(lgTp[:, tt, :],
                                        logits[:, 128 * t: 128 * (t + 1)],
                                        ids[0:41, 0:41])
                lgT = workpool.tile([128, TPS, 41], F32, tag="lgT")
                nc.scalar.activation(lgT[:, 0:nt], lgTp[:, 0:nt], ACTF.Copy)

                # ---- mask pipeline on GpSimd (exp fp32-safe w/o max) ----
                p8 = workpool.tile([128, TPS, 8], F32, tag="p8")
                nc.gpsimd.tensor_tensor(p8[:, 0:nt], lgT[:, 0:nt, 25:33],
                                        lgT[:, 0:nt, 33:41], OP.mult)
                p4 = workpool.tile([128, TPS, 4], F32, tag="p4")
                nc.gpsimd.tensor_tensor(p4[:, 0:nt], p8[:, 0:nt, 0:4],
                                        p8[:, 0:nt, 4:8], OP.mult)
                p2 = workpool.tile([128, TPS, 2], F32, tag="p2")
                nc.gpsimd.tensor_tensor(p2[:, 0:nt], p4[:, 0:nt, 0:2],
                                        p4[:, 0:nt, 2:4], OP.mult)
                i0 = workpool.tile([128, TPS], F32, tag="i0")
                nc.gpsimd.tensor_tensor(i0[:, 0:nt], p2[:, 0:nt, 0],
                                        p2[:, 0:nt, 1], OP.mult)
                ic = workpool.tile([128, TPS], F32, tag="ic")
                nc.gpsimd.tensor_scalar(ic[:, 0:nt], i0[:, 0:nt], 10.0,
                                        -10.0, OP.min, OP.max)

                mskl = workpool.tile([128, TPS, 25], F32, tag="mskl")
                nc.gpsimd.tensor_tensor(
                    mskl[:, 0:nt], lgT[:, 0:nt, 0:25],
                    ic[:, 0:nt].to_broadcast([128, nt, 25]), OP.mult)
                mexp = workpool.tile([128, TPS, 25], F32, tag="mexp")
                nc.scalar.activation(mexp[:, 0:nt], mskl[:, 0:nt], ACTF.Exp)
                msum = workpool.tile([128, TPS], F32, tag="msum")
                nc.vector.tensor_reduce(msum[:, 0:nt], mexp[:, 0:nt], AX.X,
                                        OP.add)
                mrec = workpool.tile([128, TPS], F32, tag="mrec")
                nc.vector.reciprocal(mrec[:, 0:nt], msum[:, 0:nt])
                mskn = workpool.tile([128, TPS, 25], F32, tag="mskn")
                nc.gpsimd.tensor_tensor(
                    mskn[:, 0:nt], mexp[:, 0:nt],
                    mrec[:, 0:nt].to_broadcast([128, nt, 25]), OP.mult)
                mskns[si] = mskn

            def emit_slab_back(si):
                ra, rb, ho, nr = SLABS[si]
                pos0 = 64 * ho
                nps = 64 * nr
                nt = nps // 128
                mskn = mskns[si]
                # ---- transpose mask back to channel-major (fp16) ----
                mcp = ps4.tile([25, TPS, 128], F32, tag="ps4")
                for tt in range(nt):
                    nc.tensor.transpose(mcp[:, tt, :], mskn[:, tt, :],
                                        ids[:])
                nc.scalar.activation(
                    mcm[:, pos0: pos0 + nps],
                    mcp[:, 0:nt].rearrange("k t p -> k (t p)"), ACTF.Copy)



            def emit_reassembly(row0, nrows, cname):
                # rows [row0, row0+nrows): 3-dim FD=nrows*64 APs per
                # (tap, chip); chip chains alternate so consecutive DVE
                # ops are independent. mb broadcasts are throttled by
                # the 8-slot mb pool. One bounce per chunk region keeps
                # a single mscr writer per broadcast read (sound deps).
                # single full bounce: one writer of mscr so every
                # broadcast read gets a sound dependency
                pos0, npos = 64 * row0, 64 * nrows
                accv = [accboth[:, ch, row0: row0 + nrows, :]
                        for ch in range(2)]
                nc.sync.dma_start(out=mscr[:], in_=mcm[:])
                for k in range(K5 * K5):
                    mb = mbpool.tile([128, npos], F16, tag=f"mb{cname}")
                    nc.sync.dma_start(
                        out=mb[:],
                        in_=mscr[k: k + 1,
                                 pos0: pos0 + npos].to_broadcast(
                                     [128, npos]))
                    mbs[k] = mb.rearrange("p (h w) -> p h w", h=nrows)
                for k in range(K5 * K5):
                    ky, kx = k // K5, k % K5
                    g = 2 * (ky % 2) + (kx % 2)
                    ts = []
                    for ch in range(2):
                        xsrc = xboth[:, ch, g,
                                     row0 + ky // 2: row0 + ky // 2 + nrows,
                                     kx // 2: kx // 2 + 64]
                        if k == 0:
                            nc.vector.tensor_tensor(accv[ch], xsrc,
                                                    mbs[k][:], OP.mult)
                        else:
                            t = tmppool.tile([128, nrows, 64], F16,
                                             tag=f"tp{cname}")
                            nc.vector.tensor_tensor(t[:], xsrc, mbs[k][:],
                                                    OP.mult)
                            ts.append(t)
                    for ch, t in enumerate(ts):
                        nc.vector.tensor_tensor(accv[ch], accv[ch], t[:],
                                                OP.add)

                # ---- store this chunk (fp16, host upcasts) ----
                for ch in range(2):
                    nc.scalar.dma_start(
                        out=yf[128 * ch: 128 * (ch + 1),
                               pos0: pos0 + npos],
                        in_=accboth[:, ch, row0: row0 + nrows, :].rearrange(
                            "p h w -> p (h w)"))

            # ---- pipeline: all conv blocks back-to-back (keeps the PE
            # HAM warm), then both mask-slab fronts (so the two softmax
            # chains pipeline), then the mask transposes, then one full
            # reassembly ----
            for hob in range(0, HO, 8):
                emit_conv_block(hob)
            emit_slab_front(0)
            emit_slab_front(1)
            emit_slab_back(0)
            emit_slab_back(1)
            emit_reassembly(0, HO, "a")

    nc.finalize()
    return nc


def make_core_inputs(x, w_comp, b_comp, w_enc, b_enc, w_kenc, b_kenc):
    """Full inputs -> list of 8 per-core input dicts."""
    x = np.asarray(x)
    w1 = np.asarray(w_comp).reshape(CC, C).astype(np.float64)  # [64, 256]
    we = np.asarray(w_enc)    # [25, 64, 3, 3]
    wk = np.asarray(w_kenc)   # [16, 64, 3, 3]
    w41 = np.concatenate([we, wk], axis=0).astype(np.float64)  # [41,64,3,3]
    # compose: logits-conv applied directly to x (all biases are zero)
    wq = np.einsum('kcyx,cd->kdyx', w41, w1)   # [41, 256, 3, 3]

    # stationaries per (3x3 tap, chip): wc[p, (2j+ch)*41 + k]
    # tap order must match kernel TAPS9 (grouped by subgrid 3,2,1,0)
    TAPS9_YX = [(0, 0), (0, 2), (2, 0), (2, 2),
                (0, 1), (2, 1), (1, 0), (1, 2), (1, 1)]
    wch = np.zeros((128, 18, 41), np.float64)
    for j, (ty, tx) in enumerate(TAPS9_YX):
        for ch in range(2):
            wch[:, 2 * j + ch] = wq[:, 128 * ch: 128 * (ch + 1), ty, tx].T
    wch = wch.reshape(128, 18 * 41).astype(np.float16)
    ident = np.eye(128, dtype=np.float32)

    maps = []
    for core in range(8):
        b, h = core // 2, core % 2
        start = 64 * h
        xpc = np.zeros((C, 68, 132), np.float32)
        lo, hi = start - 2, start + 66
        clo, chi = max(lo, 0), min(hi, H)
        xpc[:, clo - lo: clo - lo + (chi - clo), 2:130] = x[b, :, clo:chi, :]
        # de-interleave: g = 2*(row%2) + (col%2)
        xqc = np.empty((C, 4, GH, GW), np.float16)
        xqc[:, 0] = xpc[:, 0::2, 0::2]
        xqc[:, 1] = xpc[:, 0::2, 1::2]
        xqc[:, 2] = xpc[:, 1::2, 0::2]
        xqc[:, 3] = xpc[:, 1::2, 1::2]
        maps.append({
            "xq": xqc,
            "wc": wch,
            "ident": ident,
        })
    return maps


def assemble_output(results):
    out = np.zeros((B, C, 64, 64), np.float32)
    for core in range(8):
        b, h = core // 2, core % 2
        out[b, :, 32 * h: 32 * (h + 1), :] = results[core]["y"].astype(
            np.float32)
    return out


_NC_CACHE = []


def kernel(**inputs):
    import numpy as _np
    from concourse.bass_utils import run_bass_kernel_spmd

    maps = make_core_inputs(
        inputs["x"], inputs["w_comp"], inputs["b_comp"], inputs["w_enc"],
        inputs["b_enc"], inputs["w_kenc"], inputs["b_kenc"])
    if not _NC_CACHE:
        _NC_CACHE.append(build_nc())
    res = run_bass_kernel_spmd(_NC_CACHE[0], maps, list(range(8)))
    out = assemble_output(res.results)
    return out.astype(_np.float32)
